# revision 54
# baseline (speedup 1.0000x reference)
"""Causal self-attention (GQA + RoPE) for TRN2, sharded over 8 NeuronCores.

Sharding: tensor-parallel over heads. Each core owns 4 query heads and 1 KV
head (H=32, HKV=8 -> group size 4). Column-parallel q/k/v projections,
row-parallel o_proj; the final all-reduce over the 8 partial [T, D] outputs
happens on the host after the gather.

Layout strategy (all on-chip tensors keep the contraction dim on partitions):
  - x is uploaded pre-transposed as xT [D, T] in bf16; projections produce
    qT/kT/vT [d, t] directly in PSUM (f32 accum), rope rounds to bf16.
  - RoPE uses a host-side permutation of the head dim into [even-pairs | odd-
    pairs] halves so the pair rotation becomes: out = q*cos + swap64(q*sin'),
    where swap64 swaps 32-row halves within each 64-row head block (done with
    SBUF->SBUF DMAs) and sin' carries the sign pattern [+sin | -sin].
  - Scores are computed transposed (scoresT [s, t]); chunk PAIRS share one
    2-bank PSUM tile so a single fused exp covers 1024 columns, amortizing
    the ~293ns ACT pipe-fill. The softmax denominator comes out of the
    attn@v matmul for free via an extra ones-column on the v stationary.
  - Causal masking: above-diagonal s-blocks are skipped entirely; diagonal
    128-blocks get an affine_select (j >= i) post-exp.
  - 1/denom runs on the DVE (exact reciprocal; ACT would thrash its table
    sets against exp), is DMA-shifted to partition 0 and partition-broadcast
    to rows 0:64 for the normalize multiply.
  - o_proj PSUM eviction rides the DVE (tensor_copy to bf16) and the partial
    output is DMA'd to DRAM in bf16; the host sums the 8 partials in f64.
All matmuls run in bf16 (1 cycle/row); rel err ~1e-3 vs the 2e-2 gate.
"""

import math

import numpy as np

import concourse.bass as bass
import concourse.mybir as mybir
import concourse.tile as tile
from concourse import bacc
from concourse.masks import make_identity

D = 2048
H = 32
HKV = 8
HD = 64
T = 2048
NCORES = 8
HPC = H // NCORES        # 4 query heads per core
QC = HPC * HD            # 256 q dims per core
ROPE_BASE = 10000.0
S = 512                  # t-strip width
NSTRIP = T // S          # 4
KC = D // 128            # 16 contraction chunks

F32 = mybir.dt.float32
BF16 = mybir.dt.bfloat16

N_PROJ_OPS = 2 * KC + KC + 4   # q (2 head-pairs) + kv + v-transposes = 52
N_OPROJ_OPS = 4 * 4 * 2        # tsub x n x c = 32


def _build_kernel(debug=False):
    nc = bacc.Bacc("TRN2", target_bir_lowering=False, debug=False,
                   num_devices=NCORES)

    xT = nc.dram_tensor("xT", [D, T], BF16, kind="ExternalInput").ap()
    wqT = nc.dram_tensor("wqT", [D, QC], BF16, kind="ExternalInput").ap()
    wkvT = nc.dram_tensor("wkvT", [D, 128], BF16, kind="ExternalInput").ap()
    woT = nc.dram_tensor("woT", [QC, D], BF16, kind="ExternalInput").ap()
    cosT = nc.dram_tensor("cosT", [128, T], F32, kind="ExternalInput").ap()
    sinT = nc.dram_tensor("sinT", [128, T], F32, kind="ExternalInput").ap()
    out = nc.dram_tensor("out", [T, D], BF16, kind="ExternalOutput").ap()
    dbg = {}
    if debug:
        for nm, shp in [("d_qT0", [128, T]), ("d_qT1", [128, T]),
                        ("d_kT", [128, T]), ("d_ytps", [128, S]),
                        ("d_bc", [128, S])]:
            dbg[nm] = nc.dram_tensor(nm, shp, F32, kind="ExternalOutput").ap()

    with tile.TileContext(nc) as tc:
        with (
            tc.tile_pool(name="consts", bufs=1) as consts,
            tc.tile_pool(name="persist", bufs=1) as persist,
            tc.tile_pool(name="xa", bufs=48) as xap,
            tc.tile_pool(name="rtmp", bufs=6) as rtmp,
            tc.tile_pool(name="swp", bufs=4) as swp,
            tc.tile_pool(name="vtmp", bufs=2) as vtmp,
            tc.tile_pool(name="expp", bufs=3) as expp,
            tc.tile_pool(name="ytn", bufs=8) as ytnp,
            tc.tile_pool(name="outst", bufs=4) as outst,
            tc.tile_pool(name="dn", bufs=4) as dnp,
            tc.tile_pool(name="mm", bufs=2, space="PSUM") as mmp,
            tc.tile_pool(name="ytps", bufs=2, space="PSUM") as ytps,
            tc.tile_pool(name="pairp", bufs=2, space="PSUM") as pairp,
        ):
            # identity + warmup junk come first so their gpsimd ops are not
            # queued behind DMA descriptor generation
            ident = consts.tile([128, 128], F32)
            make_identity(nc, ident)
            identb = consts.tile([128, 128], BF16)
            nc.vector.tensor_copy(identb, ident)
            junk = consts.tile([128, 512], F32)
            nc.vector.memset(junk, 1.0)

            # ---- constants; DMA issue order interleaved per k-chunk so the
            # first projection matmul only waits on chunk 0 of wq/x ----
            wq_sb = consts.tile([128, KC, QC], BF16)
            wkv_sb = consts.tile([128, KC, 128], BF16)
            xa_strips = {}

            def load_xa(strip):
                t0 = strip * S
                xa = []
                for kc in range(KC):
                    xt = xap.tile([128, S], BF16, tag="xa", name=f"xa{strip}_{kc}")
                    nc.sync.dma_start(
                        out=xt, in_=xT[kc * 128:(kc + 1) * 128, t0:t0 + S])
                    xa.append(xt)
                xa_strips[strip] = xa

            # startup input DMAs are ISSUE-bound (~600ns per descriptor), so
            # spread them across three otherwise-idle queues
            xa0 = []
            for kc in range(KC):
                nc.scalar.dma_start(
                    out=wq_sb[:, kc, :], in_=wqT[kc * 128:(kc + 1) * 128, :])
                nc.gpsimd.dma_start(
                    out=wkv_sb[:, kc, :], in_=wkvT[kc * 128:(kc + 1) * 128, :])
                xt = xap.tile([128, S], BF16, tag="xa", name=f"xa0_{kc}")
                nc.sync.dma_start(out=xt, in_=xT[kc * 128:(kc + 1) * 128, 0:S])
                xa0.append(xt)
            xa_strips[0] = xa0
            cs_c = consts.tile([128, T], F32)
            cs_s = consts.tile([128, T], F32)
            for n in range(NSTRIP):
                nc.scalar.dma_start(
                    out=cs_c[:, n * S:(n + 1) * S],
                    in_=cosT[:, n * S:(n + 1) * S])
                nc.scalar.dma_start(
                    out=cs_s[:, n * S:(n + 1) * S],
                    in_=sinT[:, n * S:(n + 1) * S])
            wo_sb = consts.tile([128, 2, D], BF16)
            nc.scalar.dma_start(
                out=wo_sb, in_=woT.rearrange("(c p) n -> p c n", p=128))
            # second strip of x prefetched right behind the first
            load_xa(1)
            # PE warmup: sustained array activity lifts the HAM 1.2GHz cold
            # throttle while the first input DMAs land (fp32 on purpose:
            # 4 cyc/row keeps the array busy longer per instruction)
            warm_ps = pairp.tile([128, 1024], F32, tag="pair", name="warm")
            for w in range(6):
                nc.tensor.matmul(
                    warm_ps[:, 0:512], ident, junk,
                    start=True, stop=True, skip_group_check=True)

            # persistent activations
            qT = [persist.tile([128, T], BF16, tag=f"qT{i}", name=f"qT{i}")
                  for i in range(2)]
            # k duplicated on both partition halves so each q head can use
            # a stationary slice whose base partition matches its rhs base
            kT = persist.tile([128, T], BF16)
            vaug = persist.tile([128, 4 * NSTRIP, 65], BF16)
            ones_col = consts.tile([128, 4 * NSTRIP, 1], F32)
            nc.vector.memset(ones_col, 1.0)
            nc.vector.tensor_copy(vaug[:, :, 64:65], ones_col)

            def rope_q(strip, hp, pq):
                t0 = strip * S
                tsl = slice(t0, t0 + S)
                qc = rtmp.tile([128, S], BF16, tag="rtmp", name=f"qc{strip}{hp}")
                qs = rtmp.tile([128, S], BF16, tag="rtmp", name=f"qs{strip}{hp}")
                nc.vector.tensor_mul(qc, pq, cs_c[:, tsl])
                nc.vector.tensor_mul(qs, pq, cs_s[:, tsl])
                sw = swp.tile([128, S], BF16, tag="swp", name=f"sw{strip}{hp}")
                for b in range(2):
                    nc.gpsimd.dma_start(
                        out=sw[b * 64:b * 64 + 32, :],
                        in_=qs[b * 64 + 32:b * 64 + 64, :])
                    nc.gpsimd.dma_start(
                        out=sw[b * 64 + 32:b * 64 + 64, :],
                        in_=qs[b * 64:b * 64 + 32, :])
                nc.vector.tensor_add(qT[hp][:, tsl], qc, sw)

            def rope_kv(strip, pkv):
                t0 = strip * S
                tsl = slice(t0, t0 + S)
                kc_t = rtmp.tile([128, S], BF16, tag="rtmp", name=f"kc{strip}")
                ks_t = rtmp.tile([128, S], BF16, tag="rtmp", name=f"ks{strip}")
                nc.vector.tensor_mul(
                    kc_t[0:64, :], pkv[0:64, :], cs_c[0:64, tsl])
                nc.vector.tensor_mul(
                    ks_t[0:64, :], pkv[0:64, :], cs_s[0:64, tsl])
                swk = swp.tile([128, S], BF16, tag="swp", name=f"swk{strip}")
                nc.gpsimd.dma_start(out=swk[0:32, :], in_=ks_t[32:64, :])
                nc.gpsimd.dma_start(out=swk[32:64, :], in_=ks_t[0:32, :])
                nc.vector.tensor_add(
                    kT[0:64, tsl], kc_t[0:64, :], swk[0:64, :])
                nc.gpsimd.dma_start(out=kT[64:128, tsl], in_=kT[0:64, tsl])
                vt_s = vtmp.tile([128, S], BF16, tag="vtmp", name=f"vt{strip}")
                nc.vector.tensor_copy(vt_s[64:128, :], pkv[64:128, :])
                return vt_s

            def v_transpose(strip, vt_s, n):
                pt = mmp.tile([128, 64], BF16, tag="mmp", bufs=1,
                              name=f"pt{strip}{n}")
                nc.tensor.transpose(
                    pt, vt_s[64:128, n * 128:(n + 1) * 128],
                    identb[64:128, 64:128])
                nc.vector.tensor_copy(vaug[:, strip * 4 + n, 0:64], pt)

            def proj_strip0():
                """Strip 0 runs dense and DMA-paced, so all three projection
                groups interleave per x-chunk (q head-pairs in the idle pair
                banks) and finish right after the last chunk lands."""
                xa = xa_strips[0]
                pq0 = mmp.tile([128, S], F32, tag="mmp", bufs=1, name="s0pq0")
                pq1 = pairp.tile([128, 1024], F32, tag="pair", name="s0pq1")
                pkv = pairp.tile([128, 1024], F32, tag="pair", name="s0pkv")
                for kc in range(KC):
                    nc.tensor.matmul(
                        pq0, wq_sb[:, kc, 0:128], xa[kc],
                        start=(kc == 0), stop=(kc == KC - 1),
                        skip_group_check=True)
                    nc.tensor.matmul(
                        pq1[:, 0:512], wq_sb[:, kc, 128:256], xa[kc],
                        start=(kc == 0), stop=(kc == KC - 1),
                        skip_group_check=True)
                    nc.tensor.matmul(
                        pkv[:, 0:512], wkv_sb[:, kc, :], xa[kc],
                        start=(kc == 0), stop=(kc == KC - 1),
                        skip_group_check=True)
                rope_q(0, 0, pq0)
                rope_q(0, 1, pq1[:, 0:512])
                vt_s = rope_kv(0, pkv[:, 0:512])
                for n in range(4):
                    v_transpose(0, vt_s, n)

            def proj_filler(strip):
                """Yield closures, each emitting one PE op of this strip's
                q/kv projection; rope/evict DVE work rides along after the
                last matmul of each accumulation group."""
                xa = xa_strips[strip]

                for hp in range(2):
                    pq = mmp.tile([128, S], F32, tag="mmp", bufs=1,
                                   name=f"pq{strip}_{hp}")
                    for kc in range(KC):
                        def mk(hp=hp, pq=pq, kc=kc):
                            nc.tensor.matmul(
                                pq, wq_sb[:, kc, hp * 128:(hp + 1) * 128],
                                xa[kc], start=(kc == 0), stop=(kc == KC - 1))
                            if kc == KC - 1:
                                rope_q(strip, hp, pq)
                        yield mk

                pkv = mmp.tile([128, S], F32, tag="mmp", bufs=1,
                                name=f"pkv{strip}")
                state = {}
                for kc in range(KC):
                    def mk(kc=kc):
                        nc.tensor.matmul(
                            pkv, wkv_sb[:, kc, :], xa_strips[strip][kc],
                            start=(kc == 0), stop=(kc == KC - 1))
                        if kc == KC - 1:
                            state["vt_s"] = rope_kv(strip, pkv)
                    yield mk
                for n in range(4):
                    def mk(n=n):
                        v_transpose(strip, state["vt_s"], n)
                    yield mk

            def oproj_filler(strip, ytn):
                """Yield closures, each emitting one o_proj matmul; the
                eviction + store ride along after each group's stop."""
                t0 = strip * S
                for tsub in range(4):
                    trow = t0 + tsub * 128
                    for n in range(4):
                        po = mmp.tile([128, S], F32, tag="mmo", bufs=1,
                                      name=f"po{strip}{tsub}{n}")
                        for c in range(2):
                            def mk(po=po, c=c, tsub=tsub, n=n, trow=trow):
                                nc.tensor.matmul(
                                    po,
                                    ytn[c][:, tsub * 128:(tsub + 1) * 128],
                                    wo_sb[:, c, n * S:(n + 1) * S],
                                    start=(c == 0), stop=(c == 1),
                                    skip_group_check=True)
                                if c == 1:
                                    ot = outst.tile(
                                        [128, S], BF16, tag="out",
                                        name=f"ot{strip}{tsub}{n}")
                                    nc.vector.tensor_copy(ot, po)
                                    nc.sync.dma_start(
                                        out=out[trow:trow + 128,
                                                n * S:(n + 1) * S],
                                        in_=ot)
                            yield mk

            # strip 0 projection runs dense (nothing to overlap with)
            proj_strip0()

            import itertools
            ytn_strips = {}

            # filler queues drained strictly FIFO (they share single-bank
            # PSUM rings, so two open queues must never interleave), paced
            # EDF-style: enough ops per call site that every queue finishes
            # by the end of its deadline strip. Attention is slightly
            # ACT-bound per pair, so rationing PE filler across strips keeps
            # every strip PE-bound and the HAM clock-gate warm.
            # Queue: [iter, remaining, deadline_strip, min_call_index]
            queues = []
            CALLS_PER_STRIP = [HPC * (2 * (s + 1) + 2) for s in range(NSTRIP)]

            def make_queues(strip):
                if strip == 0:
                    queues.append([iter(proj_filler(1)), N_PROJ_OPS, 0, 0])
                    queues.append([iter(proj_filler(2)), N_PROJ_OPS, 1, 8])
                elif strip == 1:
                    queues.append([iter(proj_filler(3)), N_PROJ_OPS, 2, 0])
                    queues.append([iter(oproj_filler(0, ytn_strips[0])),
                                   N_OPROJ_OPS, 2, 6])
                elif strip == 2:
                    queues.append([iter(oproj_filler(1, ytn_strips[1])),
                                   N_OPROJ_OPS, 3, 6])
                elif strip == 3:
                    queues.append([iter(oproj_filler(2, ytn_strips[2])),
                                   N_OPROJ_OPS, 3, 6])

            for strip in range(NSTRIP):
                t0 = strip * S
                n_sc = (strip + 1) * 4
                n_pair = n_sc // 2
                ytn = [ytnp.tile([128, S], BF16, tag="ytn",
                                 name=f"ytn{strip}{i}") for i in range(2)]
                ytn_strips[strip] = ytn

                # x two strips ahead so projection fillers never block the
                # in-order PE on a just-issued DMA
                if strip + 2 < NSTRIP:
                    load_xa(strip + 2)
                make_queues(strip)

                n_calls = CALLS_PER_STRIP[strip]
                ci = {"i": 0}
                defib = None
                if strip == 3:
                    # keep-warm target: strip 3 has no proj fillers, so the
                    # mmp bank is free for junk matmuls that stop the HAM
                    # clock-gate from seeing an idle PE window
                    defib = mmp.tile([128, S], F32, tag="mmp", bufs=1,
                                     name="defib")

                def run_fillers(strip=strip, n_calls=n_calls, ci=ci,
                                defib=defib):
                    emitted_any = [False]
                    i = ci["i"]
                    ci["i"] += 1
                    take = 0
                    due = 0
                    for q in sorted(queues, key=lambda q: q[2]):
                        if q[3] > i:
                            continue
                        due += q[1]
                        calls_left = (n_calls - i) + sum(
                            CALLS_PER_STRIP[s] for s in range(strip + 1, q[2] + 1))
                        if calls_left > 0:
                            take = max(take, -(-due // calls_left))
                    while take > 0 and queues:
                        q = queues[0]
                        if q[1] <= 0:
                            queues.pop(0)
                            continue
                        if q[3] > i:
                            break
                        fns = list(itertools.islice(q[0], min(take, q[1])))
                        for fn in fns:
                            fn()
                            emitted_any[0] = True
                        q[1] -= len(fns)
                        take -= len(fns)
                        if q[1] <= 0 or not fns:
                            queues.pop(0)
                    if defib is not None and not emitted_any[0]:
                        nc.tensor.matmul(
                            defib, identb, ytn_strips[2][0],
                            start=True, stop=True, skip_group_check=True)

                # even heads (lo=0) first: odd heads need the kT half-dup
                # DMA, which lands a bit after the strip's k-rope
                for h in (0, 2, 1, 3):
                    hp, lo = h // 2, (h % 2) * 64
                    even = (h % 2 == 0)
                    yt_ps = ytps.tile([128, S], F32, tag="yt",
                                      name=f"yt{strip}{h}")

                    def emit_sc(P, h=h, hp=hp, lo=lo):
                        """Scores for chunk pair (2P, 2P+1) into one 2-bank
                        PSUM tile + a single fused exp."""
                        pair = pairp.tile([128, 1024], F32, tag="pair",
                                          name=f"p{strip}{h}{P}")
                        exd = expp.tile([128, 1024], BF16, tag="exp",
                                        name=f"e{strip}{h}{P}")
                        os_ = []
                        for c in range(2):
                            j = P * 2 + c
                            o = max(j * 128 - t0, 0)
                            os_.append(o)
                            nc.tensor.matmul(
                                pair[:, c * 512 + o:(c + 1) * 512],
                                kT[lo:lo + 64, j * 128:(j + 1) * 128],
                                qT[hp][lo:lo + 64, t0 + o:t0 + S],
                                start=True, stop=True)
                        # one exp over the pair; the [512:512+o1) gap holds
                        # stale psum that is exp'd but never read downstream
                        nc.scalar.activation(
                            exd[:, os_[0]:1024], pair[:, os_[0]:1024],
                            mybir.ActivationFunctionType.Exp,
                            scale=1.0 / math.sqrt(HD))
                        for c in range(2):
                            j = P * 2 + c
                            o = os_[c]
                            if j * 128 - t0 >= 0:
                                b = c * 512 + o
                                nc.gpsimd.affine_select(
                                    out=exd[:, b:b + 128],
                                    in_=exd[:, b:b + 128],
                                    pattern=[[1, 128]], base=0,
                                    channel_multiplier=-1,
                                    compare_op=mybir.AluOpType.is_ge, fill=0.0)
                        return exd, os_

                    def emit_av(P, exd, os_, yt_ps=yt_ps, n_pair=n_pair):
                        for c in range(2):
                            j = P * 2 + c
                            o = os_[c]
                            nc.tensor.matmul(
                                yt_ps[0:65, o:S], vaug[:, j, :],
                                exd[:, c * 512 + o:(c + 1) * 512],
                                start=(P == 0 and c == 0),
                                stop=(P == n_pair - 1 and c == 1),
                                skip_group_check=True)

                    prev = None
                    for P in range(n_pair):
                        cur = emit_sc(P)
                        if prev is not None:
                            emit_av(P - 1, *prev)
                            run_fillers()
                        prev = cur
                    emit_av(n_pair - 1, *prev)
                    run_fillers()
                    # normalize: exact 1/denom on the DVE. The denom row is
                    # DMA-spread across 64 lanes first: DVE reciprocal cost
                    # scales with free size (~3.4us on [1,512], ~0.2us on
                    # [64,8]), and ACT would thrash table sets against exp.
                    drow = dnp.tile([128, S], F32, tag="drow", bufs=2,
                                    name=f"drow{strip}{h}")
                    nc.vector.tensor_copy(drow[64:65, :], yt_ps[64:65, :])
                    rs = dnp.tile([128, 8], F32, tag="rs", bufs=2,
                                  name=f"rs{strip}{h}")
                    nc.gpsimd.dma_start(out=rs[0:64, :], in_=drow[64:65, :])
                    rs2 = dnp.tile([128, 8], F32, tag="rs2", bufs=2,
                                   name=f"rs2{strip}{h}")
                    nc.vector.reciprocal(rs2[0:64, :], rs[0:64, :])
                    dn_f = dnp.tile([128, S], F32, tag="dnr", bufs=2,
                                    name=f"dnr{strip}{h}")
                    nc.gpsimd.dma_start(out=dn_f[0:1, :], in_=rs2[0:64, :])
                    nc.gpsimd.dma_start(out=dn_f[32:33, :], in_=dn_f[0:1, :])
                    run_fillers()
                    # broadcast p0/p32 across their 32-partition quadrants
                    # with a stream_shuffle (hw DVE op; the gpsimd ucode
                    # partition_broadcast showed a timing-dependent race)
                    bc_t = dnp.tile([128, S], F32, tag="dn", name=f"bc{strip}{h}")
                    nc.vector.stream_shuffle(
                        bc_t[0:64, :], dn_f[0:64, :], mask=[0] * 32)
                    if debug and strip == 0 and h == 0:
                        yd = dnp.tile([128, S], F32, tag="dn", name="yd")
                        nc.vector.tensor_copy(yd, yt_ps)
                        nc.sync.dma_start(out=dbg["d_ytps"], in_=yd)
                        nc.sync.dma_start(out=dbg["d_bc"], in_=bc_t)
                    if even:
                        nc.vector.tensor_mul(
                            ytn[hp][0:64, :], yt_ps[0:64, :], bc_t[0:64, :])
                    else:
                        ntmp = dnp.tile([128, S], BF16, tag="ntmp", bufs=2,
                                        name=f"nt{strip}{h}")
                        nc.vector.tensor_mul(
                            ntmp[0:64, :], yt_ps[0:64, :], bc_t[0:64, :])
                        nc.gpsimd.dma_start(
                            out=ytn[hp][64:128, :], in_=ntmp[0:64, :])

                if debug and strip == 0:
                    nc.sync.dma_start(out=dbg["d_qT0"][:, 0:1024],
                                      in_=qT[0].bitcast(F32)[:, 0:1024])
                    nc.sync.dma_start(out=dbg["d_qT1"][:, 0:1024],
                                      in_=qT[1].bitcast(F32)[:, 0:1024])
                    nc.sync.dma_start(out=dbg["d_kT"][:, 0:1024],
                                      in_=kT.bitcast(F32)[:, 0:1024])

                # drain queues whose deadline is this strip (FIFO order)
                while queues and queues[0][2] <= strip:
                    q = queues.pop(0)
                    for fn in q[0]:
                        fn()

            # last strip's o_proj runs dense at the tail
            for fn in oproj_filler(NSTRIP - 1, ytn_strips[NSTRIP - 1]):
                fn()

    nc.compile()
    return nc


_NC_CACHE = None


def _get_nc():
    global _NC_CACHE
    if _NC_CACHE is None:
        _NC_CACHE = _build_kernel()
    return _NC_CACHE


def _prep_inputs(x, wq, wk, wv, wo):
    """Host-side shard + layout prep. Returns per-core input maps."""
    import ml_dtypes
    BF = ml_dtypes.bfloat16
    x = np.asarray(x, dtype=np.float32).reshape(T, D)
    wq = np.asarray(wq, dtype=np.float32)
    wk = np.asarray(wk, dtype=np.float32)
    wv = np.asarray(wv, dtype=np.float32)
    wo = np.asarray(wo, dtype=np.float32)

    xT_b = np.ascontiguousarray(x.T).astype(BF)

    # head-dim permutation for rope: [even pair comps | odd pair comps]
    perm = np.concatenate([np.arange(0, HD, 2), np.arange(1, HD, 2)])

    # rope tables in the [d, t] layout
    theta = 1.0 / ROPE_BASE ** (np.arange(0, HD, 2, dtype=np.float64) / HD)
    ang = np.arange(T, dtype=np.float64)[None, :] * theta[:, None]  # [32, T]
    cos_blk = np.cos(ang).astype(np.float32)
    sin_blk = np.sin(ang).astype(np.float32)
    cosT = np.tile(np.concatenate([cos_blk, cos_blk], 0), (2, 1))
    sinT = np.tile(np.concatenate([sin_blk, -sin_blk], 0), (2, 1))
    cosT = np.ascontiguousarray(cosT)
    sinT = np.ascontiguousarray(sinT)

    in_maps = []
    for c in range(NCORES):
        wq_c = wq[c * QC:(c + 1) * QC].reshape(HPC, HD, D)[:, perm, :]
        wq_c = wq_c.reshape(QC, D)
        wk_c = wk[c * HD:(c + 1) * HD][perm, :]
        wv_c = wv[c * HD:(c + 1) * HD]
        wkv_c = np.concatenate([wk_c, wv_c], axis=0)          # [128, D]
        wo_c = wo[:, c * QC:(c + 1) * QC]                      # [D, QC]
        in_maps.append({
            "xT": xT_b,
            "wqT": np.ascontiguousarray(wq_c.T).astype(BF),
            "wkvT": np.ascontiguousarray(wkv_c.T).astype(BF),
            "woT": np.ascontiguousarray(wo_c.T).astype(BF),
            "cosT": cosT,
            "sinT": sinT,
        })
    return in_maps


def kernel(x, wq, wk, wv, wo):
    from concourse.bass_utils import run_bass_kernel_spmd

    nc = _get_nc()
    in_maps = _prep_inputs(x, wq, wk, wv, wo)
    res = run_bass_kernel_spmd(nc, in_maps, core_ids=list(range(NCORES)))
    acc = np.zeros((T, D), dtype=np.float64)
    for c in range(NCORES):
        acc += res.results[c]["out"].astype(np.float64)
    return acc.astype(np.float32).reshape(1, T, D)


# revision 55
# speedup vs baseline: 1.0272x; 1.0272x over previous
"""Causal self-attention (GQA + RoPE) for TRN2, sharded over 8 NeuronCores.

Sharding: tensor-parallel over heads. Each core owns 4 query heads and 1 KV
head (H=32, HKV=8 -> group size 4). Column-parallel q/k/v projections,
row-parallel o_proj; the final all-reduce over the 8 partial [T, D] outputs
happens on the host after the gather.

Layout strategy (all on-chip tensors keep the contraction dim on partitions):
  - x is uploaded pre-transposed as xT [D, T] in bf16; projections produce
    qT/kT/vT [d, t] directly in PSUM (f32 accum), rope rounds to bf16.
  - RoPE uses a host-side permutation of the head dim into [even-pairs | odd-
    pairs] halves so the pair rotation becomes: out = q*cos + swap64(q*sin'),
    where swap64 swaps 32-row halves within each 64-row head block (done with
    SBUF->SBUF DMAs) and sin' carries the sign pattern [+sin | -sin].
  - Scores are computed transposed (scoresT [s, t]); chunk PAIRS share one
    2-bank PSUM tile so a single fused exp covers 1024 columns, amortizing
    the ~293ns ACT pipe-fill. The softmax denominator comes out of the
    attn@v matmul for free via an extra ones-column on the v stationary.
  - Causal masking: above-diagonal s-blocks are skipped entirely; diagonal
    128-blocks get an affine_select (j >= i) post-exp.
  - 1/denom runs on the DVE (exact reciprocal; ACT would thrash its table
    sets against exp), is DMA-shifted to partition 0 and partition-broadcast
    to rows 0:64 for the normalize multiply.
  - o_proj PSUM eviction rides the DVE (tensor_copy to bf16) and the partial
    output is DMA'd to DRAM in bf16; the host sums the 8 partials in f64.
All matmuls run in bf16 (1 cycle/row); rel err ~1e-3 vs the 2e-2 gate.
"""

import math

import numpy as np

import concourse.bass as bass
import concourse.mybir as mybir
import concourse.tile as tile
from concourse import bacc
from concourse.masks import make_identity

D = 2048
H = 32
HKV = 8
HD = 64
T = 2048
NCORES = 8
HPC = H // NCORES        # 4 query heads per core
QC = HPC * HD            # 256 q dims per core
ROPE_BASE = 10000.0
S = 512                  # t-strip width
NSTRIP = T // S          # 4
KC = D // 128            # 16 contraction chunks

F32 = mybir.dt.float32
BF16 = mybir.dt.bfloat16

N_PROJ_OPS = 2 * KC + KC + 4   # q (2 head-pairs) + kv + v-transposes = 52
N_OPROJ_OPS = 4 * 4 * 2        # tsub x n x c = 32


def _build_kernel(debug=False):
    nc = bacc.Bacc("TRN2", target_bir_lowering=False, debug=False,
                   num_devices=NCORES)

    xT = nc.dram_tensor("xT", [D, T], BF16, kind="ExternalInput").ap()
    wqT = nc.dram_tensor("wqT", [D, QC], BF16, kind="ExternalInput").ap()
    wkvT = nc.dram_tensor("wkvT", [D, 128], BF16, kind="ExternalInput").ap()
    woT = nc.dram_tensor("woT", [QC, D], BF16, kind="ExternalInput").ap()
    cosT = nc.dram_tensor("cosT", [128, T], F32, kind="ExternalInput").ap()
    sinT = nc.dram_tensor("sinT", [128, T], F32, kind="ExternalInput").ap()
    out = nc.dram_tensor("out", [T, D], BF16, kind="ExternalOutput").ap()
    dbg = {}
    if debug:
        for nm, shp in [("d_qT0", [128, T]), ("d_qT1", [128, T]),
                        ("d_kT", [128, T]), ("d_ytps", [128, S]),
                        ("d_bc", [128, S])]:
            dbg[nm] = nc.dram_tensor(nm, shp, F32, kind="ExternalOutput").ap()

    with tile.TileContext(nc) as tc:
        with (
            tc.tile_pool(name="consts", bufs=1) as consts,
            tc.tile_pool(name="persist", bufs=1) as persist,
            tc.tile_pool(name="xa", bufs=48) as xap,
            tc.tile_pool(name="rtmp", bufs=6) as rtmp,
            tc.tile_pool(name="swp", bufs=4) as swp,
            tc.tile_pool(name="vtmp", bufs=2) as vtmp,
            tc.tile_pool(name="expp", bufs=3) as expp,
            tc.tile_pool(name="ytn", bufs=8) as ytnp,
            tc.tile_pool(name="outst", bufs=4) as outst,
            tc.tile_pool(name="dn", bufs=4) as dnp,
            tc.tile_pool(name="mm", bufs=2, space="PSUM") as mmp,
            tc.tile_pool(name="ytps", bufs=2, space="PSUM") as ytps,
            tc.tile_pool(name="pairp", bufs=2, space="PSUM") as pairp,
        ):
            # identity + warmup junk come first so their gpsimd ops are not
            # queued behind DMA descriptor generation
            ident = consts.tile([128, 128], F32)
            make_identity(nc, ident)
            identb = consts.tile([128, 128], BF16)
            nc.vector.tensor_copy(identb, ident)
            junk = consts.tile([128, 512], F32)
            nc.vector.memset(junk, 1.0)

            # ---- constants; DMA issue order interleaved per k-chunk so the
            # first projection matmul only waits on chunk 0 of wq/x ----
            wq_sb = consts.tile([128, KC, QC], BF16)
            wkv_sb = consts.tile([128, KC, 128], BF16)
            xa_strips = {}

            def load_xa(strip):
                t0 = strip * S
                xa = []
                for kc in range(KC):
                    xt = xap.tile([128, S], BF16, tag="xa", name=f"xa{strip}_{kc}")
                    nc.sync.dma_start(
                        out=xt, in_=xT[kc * 128:(kc + 1) * 128, t0:t0 + S])
                    xa.append(xt)
                xa_strips[strip] = xa

            # startup input DMAs are ISSUE-bound (~600ns per descriptor), so
            # spread them across three otherwise-idle queues
            xa0 = []
            for kc in range(KC):
                nc.scalar.dma_start(
                    out=wq_sb[:, kc, :], in_=wqT[kc * 128:(kc + 1) * 128, :])
                nc.gpsimd.dma_start(
                    out=wkv_sb[:, kc, :], in_=wkvT[kc * 128:(kc + 1) * 128, :])
                xt = xap.tile([128, S], BF16, tag="xa", name=f"xa0_{kc}")
                nc.sync.dma_start(out=xt, in_=xT[kc * 128:(kc + 1) * 128, 0:S])
                xa0.append(xt)
            xa_strips[0] = xa0
            cs_c = consts.tile([128, T], F32)
            cs_s = consts.tile([128, T], F32)
            for n in range(NSTRIP):
                nc.scalar.dma_start(
                    out=cs_c[:, n * S:(n + 1) * S],
                    in_=cosT[:, n * S:(n + 1) * S])
                nc.scalar.dma_start(
                    out=cs_s[:, n * S:(n + 1) * S],
                    in_=sinT[:, n * S:(n + 1) * S])
            wo_sb = consts.tile([128, 2, D], BF16)
            nc.scalar.dma_start(
                out=wo_sb, in_=woT.rearrange("(c p) n -> p c n", p=128))
            # second strip of x prefetched right behind the first
            load_xa(1)
            # PE warmup: sustained array activity lifts the HAM 1.2GHz cold
            # throttle while the first input DMAs land (fp32 on purpose:
            # 4 cyc/row keeps the array busy longer per instruction)
            warm_ps = pairp.tile([128, 1024], F32, tag="pair", name="warm")
            for w in range(6):
                nc.tensor.matmul(
                    warm_ps[:, 0:512], ident, junk,
                    start=True, stop=True, skip_group_check=True)

            # persistent activations
            qT = [persist.tile([128, T], BF16, tag=f"qT{i}", name=f"qT{i}")
                  for i in range(2)]
            # k duplicated on both partition halves so each q head can use
            # a stationary slice whose base partition matches its rhs base
            kT = persist.tile([128, T], BF16)
            vaug = persist.tile([128, 4 * NSTRIP, 65], BF16)
            ones_col = consts.tile([128, 4 * NSTRIP, 1], F32)
            nc.vector.memset(ones_col, 1.0)
            nc.vector.tensor_copy(vaug[:, :, 64:65], ones_col)

            def rope_q(strip, hp, pq):
                t0 = strip * S
                tsl = slice(t0, t0 + S)
                qc = rtmp.tile([128, S], BF16, tag="rtmp", name=f"qc{strip}{hp}")
                qs = rtmp.tile([128, S], BF16, tag="rtmp", name=f"qs{strip}{hp}")
                nc.vector.tensor_mul(qc, pq, cs_c[:, tsl])
                nc.vector.tensor_mul(qs, pq, cs_s[:, tsl])
                sw = swp.tile([128, S], BF16, tag="swp", name=f"sw{strip}{hp}")
                for b in range(2):
                    nc.gpsimd.dma_start(
                        out=sw[b * 64:b * 64 + 32, :],
                        in_=qs[b * 64 + 32:b * 64 + 64, :])
                    nc.gpsimd.dma_start(
                        out=sw[b * 64 + 32:b * 64 + 64, :],
                        in_=qs[b * 64:b * 64 + 32, :])
                nc.vector.tensor_add(qT[hp][:, tsl], qc, sw)

            def rope_kv(strip, pkv):
                t0 = strip * S
                tsl = slice(t0, t0 + S)
                kc_t = rtmp.tile([128, S], BF16, tag="rtmp", name=f"kc{strip}")
                ks_t = rtmp.tile([128, S], BF16, tag="rtmp", name=f"ks{strip}")
                nc.vector.tensor_mul(
                    kc_t[0:64, :], pkv[0:64, :], cs_c[0:64, tsl])
                nc.vector.tensor_mul(
                    ks_t[0:64, :], pkv[0:64, :], cs_s[0:64, tsl])
                swk = swp.tile([128, S], BF16, tag="swp", name=f"swk{strip}")
                nc.gpsimd.dma_start(out=swk[0:32, :], in_=ks_t[32:64, :])
                nc.gpsimd.dma_start(out=swk[32:64, :], in_=ks_t[0:32, :])
                nc.vector.tensor_add(
                    kT[0:64, tsl], kc_t[0:64, :], swk[0:64, :])
                nc.gpsimd.dma_start(out=kT[64:128, tsl], in_=kT[0:64, tsl])
                vt_s = vtmp.tile([128, S], BF16, tag="vtmp", name=f"vt{strip}")
                nc.vector.tensor_copy(vt_s[64:128, :], pkv[64:128, :])
                return vt_s

            def v_transpose(strip, vt_s, n):
                pt = mmp.tile([128, 64], BF16, tag="mmp", bufs=1,
                              name=f"pt{strip}{n}")
                nc.tensor.transpose(
                    pt, vt_s[64:128, n * 128:(n + 1) * 128],
                    identb[64:128, 64:128])
                nc.vector.tensor_copy(vaug[:, strip * 4 + n, 0:64], pt)

            def proj_strip0():
                """Strip 0 runs dense and DMA-paced, so all three projection
                groups interleave per x-chunk (q head-pairs in the idle pair
                banks) and finish right after the last chunk lands."""
                xa = xa_strips[0]
                pq0 = mmp.tile([128, S], F32, tag="mmp", bufs=1, name="s0pq0")
                pq1 = pairp.tile([128, 1024], F32, tag="pair", name="s0pq1")
                pkv = pairp.tile([128, 1024], F32, tag="pair", name="s0pkv")
                for kc in range(KC):
                    nc.tensor.matmul(
                        pq0, wq_sb[:, kc, 0:128], xa[kc],
                        start=(kc == 0), stop=(kc == KC - 1),
                        skip_group_check=True)
                    nc.tensor.matmul(
                        pq1[:, 0:512], wq_sb[:, kc, 128:256], xa[kc],
                        start=(kc == 0), stop=(kc == KC - 1),
                        skip_group_check=True)
                    nc.tensor.matmul(
                        pkv[:, 0:512], wkv_sb[:, kc, :], xa[kc],
                        start=(kc == 0), stop=(kc == KC - 1),
                        skip_group_check=True)
                rope_q(0, 0, pq0)
                rope_q(0, 1, pq1[:, 0:512])
                vt_s = rope_kv(0, pkv[:, 0:512])
                for n in range(4):
                    v_transpose(0, vt_s, n)

            def proj_filler(strip):
                """Yield closures, each emitting one PE op of this strip's
                q/kv projection; rope/evict DVE work rides along after the
                last matmul of each accumulation group."""
                xa = xa_strips[strip]

                for hp in range(2):
                    pq = mmp.tile([128, S], F32, tag="mmp", bufs=1,
                                   name=f"pq{strip}_{hp}")
                    for kc in range(KC):
                        def mk(hp=hp, pq=pq, kc=kc):
                            nc.tensor.matmul(
                                pq, wq_sb[:, kc, hp * 128:(hp + 1) * 128],
                                xa[kc], start=(kc == 0), stop=(kc == KC - 1))
                            if kc == KC - 1:
                                rope_q(strip, hp, pq)
                        yield mk

                pkv = mmp.tile([128, S], F32, tag="mmp", bufs=1,
                                name=f"pkv{strip}")
                state = {}
                for kc in range(KC):
                    def mk(kc=kc):
                        nc.tensor.matmul(
                            pkv, wkv_sb[:, kc, :], xa_strips[strip][kc],
                            start=(kc == 0), stop=(kc == KC - 1))
                        if kc == KC - 1:
                            state["vt_s"] = rope_kv(strip, pkv)
                    yield mk
                for n in range(4):
                    def mk(n=n):
                        v_transpose(strip, state["vt_s"], n)
                    yield mk

            def oproj_filler(strip, ytn):
                """Yield closures, each emitting one o_proj matmul; the
                eviction + store ride along after each group's stop."""
                t0 = strip * S
                for tsub in range(4):
                    trow = t0 + tsub * 128
                    for n in range(4):
                        po = mmp.tile([128, S], F32, tag="mmo", bufs=1,
                                      name=f"po{strip}{tsub}{n}")
                        for c in range(2):
                            def mk(po=po, c=c, tsub=tsub, n=n, trow=trow):
                                nc.tensor.matmul(
                                    po,
                                    ytn[c][:, tsub * 128:(tsub + 1) * 128],
                                    wo_sb[:, c, n * S:(n + 1) * S],
                                    start=(c == 0), stop=(c == 1),
                                    skip_group_check=True)
                                if c == 1:
                                    ot = outst.tile(
                                        [128, S], BF16, tag="out",
                                        name=f"ot{strip}{tsub}{n}")
                                    nc.vector.tensor_copy(ot, po)
                                    nc.sync.dma_start(
                                        out=out[trow:trow + 128,
                                                n * S:(n + 1) * S],
                                        in_=ot)
                            yield mk

            # strip 0 projection runs dense (nothing to overlap with)
            proj_strip0()

            import itertools
            ytn_strips = {}

            # filler queues drained strictly FIFO (they share single-bank
            # PSUM rings, so two open queues must never interleave), paced
            # EDF-style: enough ops per call site that every queue finishes
            # by the end of its deadline strip. Attention is slightly
            # ACT-bound per pair, so rationing PE filler across strips keeps
            # every strip PE-bound and the HAM clock-gate warm.
            # Queue: [iter, remaining, deadline_strip, min_call_index]
            queues = []
            CALLS_PER_STRIP = [HPC * (2 * (s + 1) + 2) for s in range(NSTRIP)]

            def make_queues(strip):
                if strip == 0:
                    queues.append([iter(proj_filler(1)), N_PROJ_OPS, 0, 0])
                    queues.append([iter(proj_filler(2)), N_PROJ_OPS, 1, 8])
                elif strip == 1:
                    queues.append([iter(proj_filler(3)), N_PROJ_OPS, 2, 0])
                    queues.append([iter(oproj_filler(0, ytn_strips[0])),
                                   N_OPROJ_OPS, 2, 6])
                elif strip == 2:
                    queues.append([iter(oproj_filler(1, ytn_strips[1])),
                                   N_OPROJ_OPS, 3, 6])
                elif strip == 3:
                    queues.append([iter(oproj_filler(2, ytn_strips[2])),
                                   N_OPROJ_OPS, 3, 6])

            for strip in range(NSTRIP):
                t0 = strip * S
                n_sc = (strip + 1) * 4
                n_pair = n_sc // 2
                ytn = [ytnp.tile([128, S], BF16, tag="ytn",
                                 name=f"ytn{strip}{i}") for i in range(2)]
                ytn_strips[strip] = ytn

                # x two strips ahead so projection fillers never block the
                # in-order PE on a just-issued DMA
                if strip + 2 < NSTRIP:
                    load_xa(strip + 2)
                make_queues(strip)

                n_calls = CALLS_PER_STRIP[strip]
                ci = {"i": 0}

                def run_fillers(strip=strip, n_calls=n_calls, ci=ci):
                    i = ci["i"]
                    ci["i"] += 1
                    take = 0
                    due = 0
                    for q in sorted(queues, key=lambda q: q[2]):
                        if q[3] > i:
                            continue
                        due += q[1]
                        calls_left = (n_calls - i) + sum(
                            CALLS_PER_STRIP[s] for s in range(strip + 1, q[2] + 1))
                        if calls_left > 0:
                            take = max(take, -(-due // calls_left))
                    while take > 0 and queues:
                        q = queues[0]
                        if q[1] <= 0:
                            queues.pop(0)
                            continue
                        if q[3] > i:
                            break
                        fns = list(itertools.islice(q[0], min(take, q[1])))
                        for fn in fns:
                            fn()
                        q[1] -= len(fns)
                        take -= len(fns)
                        if q[1] <= 0 or not fns:
                            queues.pop(0)

                # even heads (lo=0) first: odd heads need the kT half-dup
                # DMA, which lands a bit after the strip's k-rope
                for h in (0, 2, 1, 3):
                    hp, lo = h // 2, (h % 2) * 64
                    even = (h % 2 == 0)
                    yt_ps = ytps.tile([128, S], F32, tag="yt",
                                      name=f"yt{strip}{h}")

                    def emit_sc(P, h=h, hp=hp, lo=lo):
                        """Scores for chunk pair (2P, 2P+1) into one 2-bank
                        PSUM tile + a single fused exp."""
                        pair = pairp.tile([128, 1024], F32, tag="pair",
                                          name=f"p{strip}{h}{P}")
                        exd = expp.tile([128, 1024], BF16, tag="exp",
                                        name=f"e{strip}{h}{P}")
                        os_ = []
                        for c in range(2):
                            j = P * 2 + c
                            o = max(j * 128 - t0, 0)
                            os_.append(o)
                            nc.tensor.matmul(
                                pair[:, c * 512 + o:(c + 1) * 512],
                                kT[lo:lo + 64, j * 128:(j + 1) * 128],
                                qT[hp][lo:lo + 64, t0 + o:t0 + S],
                                start=True, stop=True)
                        # one exp over the pair; the [512:512+o1) gap holds
                        # stale psum that is exp'd but never read downstream
                        nc.scalar.activation(
                            exd[:, os_[0]:1024], pair[:, os_[0]:1024],
                            mybir.ActivationFunctionType.Exp,
                            scale=1.0 / math.sqrt(HD))
                        for c in range(2):
                            j = P * 2 + c
                            o = os_[c]
                            if j * 128 - t0 >= 0:
                                b = c * 512 + o
                                nc.gpsimd.affine_select(
                                    out=exd[:, b:b + 128],
                                    in_=exd[:, b:b + 128],
                                    pattern=[[1, 128]], base=0,
                                    channel_multiplier=-1,
                                    compare_op=mybir.AluOpType.is_ge, fill=0.0)
                        return exd, os_

                    def emit_av(P, exd, os_, yt_ps=yt_ps, n_pair=n_pair):
                        for c in range(2):
                            j = P * 2 + c
                            o = os_[c]
                            nc.tensor.matmul(
                                yt_ps[0:65, o:S], vaug[:, j, :],
                                exd[:, c * 512 + o:(c + 1) * 512],
                                start=(P == 0 and c == 0),
                                stop=(P == n_pair - 1 and c == 1),
                                skip_group_check=True)

                    prev = None
                    for P in range(n_pair):
                        cur = emit_sc(P)
                        if prev is not None:
                            emit_av(P - 1, *prev)
                            run_fillers()
                        prev = cur
                    emit_av(n_pair - 1, *prev)
                    run_fillers()
                    # normalize: exact 1/denom on the DVE. The denom row is
                    # DMA-spread across 64 lanes first: DVE reciprocal cost
                    # scales with free size (~3.4us on [1,512], ~0.2us on
                    # [64,8]), and ACT would thrash table sets against exp.
                    drow = dnp.tile([128, S], F32, tag="drow", bufs=2,
                                    name=f"drow{strip}{h}")
                    nc.vector.tensor_copy(drow[64:65, :], yt_ps[64:65, :])
                    rs = dnp.tile([128, 8], F32, tag="rs", bufs=2,
                                  name=f"rs{strip}{h}")
                    nc.gpsimd.dma_start(out=rs[0:64, :], in_=drow[64:65, :])
                    rs2 = dnp.tile([128, 8], F32, tag="rs2", bufs=2,
                                   name=f"rs2{strip}{h}")
                    nc.vector.reciprocal(rs2[0:64, :], rs[0:64, :])
                    dn_f = dnp.tile([128, S], F32, tag="dnr", bufs=2,
                                    name=f"dnr{strip}{h}")
                    nc.gpsimd.dma_start(out=dn_f[0:1, :], in_=rs2[0:64, :])
                    nc.gpsimd.dma_start(out=dn_f[32:33, :], in_=rs2[0:64, :])
                    run_fillers()
                    # broadcast p0/p32 across their 32-partition quadrants
                    # with a stream_shuffle (hw DVE op; the gpsimd ucode
                    # partition_broadcast showed a timing-dependent race)
                    bc_t = dnp.tile([128, S], F32, tag="dn", name=f"bc{strip}{h}")
                    nc.vector.stream_shuffle(
                        bc_t[0:64, :], dn_f[0:64, :], mask=[0] * 32)
                    if debug and strip == 0 and h == 0:
                        yd = dnp.tile([128, S], F32, tag="dn", name="yd")
                        nc.vector.tensor_copy(yd, yt_ps)
                        nc.sync.dma_start(out=dbg["d_ytps"], in_=yd)
                        nc.sync.dma_start(out=dbg["d_bc"], in_=bc_t)
                    if even:
                        nc.vector.tensor_mul(
                            ytn[hp][0:64, :], yt_ps[0:64, :], bc_t[0:64, :])
                    else:
                        ntmp = dnp.tile([128, S], BF16, tag="ntmp", bufs=2,
                                        name=f"nt{strip}{h}")
                        nc.vector.tensor_mul(
                            ntmp[0:64, :], yt_ps[0:64, :], bc_t[0:64, :])
                        nc.gpsimd.dma_start(
                            out=ytn[hp][64:128, :], in_=ntmp[0:64, :])

                if debug and strip == 0:
                    nc.sync.dma_start(out=dbg["d_qT0"][:, 0:1024],
                                      in_=qT[0].bitcast(F32)[:, 0:1024])
                    nc.sync.dma_start(out=dbg["d_qT1"][:, 0:1024],
                                      in_=qT[1].bitcast(F32)[:, 0:1024])
                    nc.sync.dma_start(out=dbg["d_kT"][:, 0:1024],
                                      in_=kT.bitcast(F32)[:, 0:1024])

                # drain queues whose deadline is this strip (FIFO order)
                while queues and queues[0][2] <= strip:
                    q = queues.pop(0)
                    for fn in q[0]:
                        fn()

            # last strip's o_proj runs dense at the tail
            for fn in oproj_filler(NSTRIP - 1, ytn_strips[NSTRIP - 1]):
                fn()

    nc.compile()
    return nc


_NC_CACHE = None


def _get_nc():
    global _NC_CACHE
    if _NC_CACHE is None:
        _NC_CACHE = _build_kernel()
    return _NC_CACHE


def _prep_inputs(x, wq, wk, wv, wo):
    """Host-side shard + layout prep. Returns per-core input maps."""
    import ml_dtypes
    BF = ml_dtypes.bfloat16
    x = np.asarray(x, dtype=np.float32).reshape(T, D)
    wq = np.asarray(wq, dtype=np.float32)
    wk = np.asarray(wk, dtype=np.float32)
    wv = np.asarray(wv, dtype=np.float32)
    wo = np.asarray(wo, dtype=np.float32)

    xT_b = np.ascontiguousarray(x.T).astype(BF)

    # head-dim permutation for rope: [even pair comps | odd pair comps]
    perm = np.concatenate([np.arange(0, HD, 2), np.arange(1, HD, 2)])

    # rope tables in the [d, t] layout
    theta = 1.0 / ROPE_BASE ** (np.arange(0, HD, 2, dtype=np.float64) / HD)
    ang = np.arange(T, dtype=np.float64)[None, :] * theta[:, None]  # [32, T]
    cos_blk = np.cos(ang).astype(np.float32)
    sin_blk = np.sin(ang).astype(np.float32)
    cosT = np.tile(np.concatenate([cos_blk, cos_blk], 0), (2, 1))
    sinT = np.tile(np.concatenate([sin_blk, -sin_blk], 0), (2, 1))
    cosT = np.ascontiguousarray(cosT)
    sinT = np.ascontiguousarray(sinT)

    in_maps = []
    for c in range(NCORES):
        wq_c = wq[c * QC:(c + 1) * QC].reshape(HPC, HD, D)[:, perm, :]
        wq_c = wq_c.reshape(QC, D)
        wk_c = wk[c * HD:(c + 1) * HD][perm, :]
        wv_c = wv[c * HD:(c + 1) * HD]
        wkv_c = np.concatenate([wk_c, wv_c], axis=0)          # [128, D]
        wo_c = wo[:, c * QC:(c + 1) * QC]                      # [D, QC]
        in_maps.append({
            "xT": xT_b,
            "wqT": np.ascontiguousarray(wq_c.T).astype(BF),
            "wkvT": np.ascontiguousarray(wkv_c.T).astype(BF),
            "woT": np.ascontiguousarray(wo_c.T).astype(BF),
            "cosT": cosT,
            "sinT": sinT,
        })
    return in_maps


def kernel(x, wq, wk, wv, wo):
    from concourse.bass_utils import run_bass_kernel_spmd

    nc = _get_nc()
    in_maps = _prep_inputs(x, wq, wk, wv, wo)
    res = run_bass_kernel_spmd(nc, in_maps, core_ids=list(range(NCORES)))
    acc = np.zeros((T, D), dtype=np.float64)
    for c in range(NCORES):
        acc += res.results[c]["out"].astype(np.float64)
    return acc.astype(np.float32).reshape(1, T, D)


# revision 56
# speedup vs baseline: 1.0329x; 1.0055x over previous
"""Causal self-attention (GQA + RoPE) for TRN2, sharded over 8 NeuronCores.

Sharding: tensor-parallel over heads. Each core owns 4 query heads and 1 KV
head (H=32, HKV=8 -> group size 4). Column-parallel q/k/v projections,
row-parallel o_proj; the final all-reduce over the 8 partial [T, D] outputs
happens on the host after the gather.

Layout strategy (all on-chip tensors keep the contraction dim on partitions):
  - x is uploaded pre-transposed as xT [D, T] in bf16; projections produce
    qT/kT/vT [d, t] directly in PSUM (f32 accum), rope rounds to bf16.
  - RoPE uses a host-side permutation of the head dim into [even-pairs | odd-
    pairs] halves so the pair rotation becomes: out = q*cos + swap64(q*sin'),
    where swap64 swaps 32-row halves within each 64-row head block (done with
    SBUF->SBUF DMAs) and sin' carries the sign pattern [+sin | -sin].
  - Scores are computed transposed (scoresT [s, t]); chunk PAIRS share one
    2-bank PSUM tile so a single fused exp covers 1024 columns, amortizing
    the ~293ns ACT pipe-fill. The softmax denominator comes out of the
    attn@v matmul for free via an extra ones-column on the v stationary.
  - Causal masking: above-diagonal s-blocks are skipped entirely; diagonal
    128-blocks get an affine_select (j >= i) post-exp.
  - 1/denom runs on the DVE (exact reciprocal; ACT would thrash its table
    sets against exp), is DMA-shifted to partition 0 and partition-broadcast
    to rows 0:64 for the normalize multiply.
  - o_proj PSUM eviction rides the DVE (tensor_copy to bf16) and the partial
    output is DMA'd to DRAM in bf16; the host sums the 8 partials in f64.
All matmuls run in bf16 (1 cycle/row); rel err ~1e-3 vs the 2e-2 gate.
"""

import math

import numpy as np

import concourse.bass as bass
import concourse.mybir as mybir
import concourse.tile as tile
from concourse import bacc
from concourse.masks import make_identity

D = 2048
H = 32
HKV = 8
HD = 64
T = 2048
NCORES = 8
HPC = H // NCORES        # 4 query heads per core
QC = HPC * HD            # 256 q dims per core
ROPE_BASE = 10000.0
S = 512                  # t-strip width
NSTRIP = T // S          # 4
KC = D // 128            # 16 contraction chunks

F32 = mybir.dt.float32
BF16 = mybir.dt.bfloat16

N_PROJ_OPS = 2 * KC + KC + 4   # q (2 head-pairs) + kv + v-transposes = 52
N_OPROJ_OPS = 4 * 4 * 2        # tsub x n x c = 32


def _build_kernel(debug=False):
    nc = bacc.Bacc("TRN2", target_bir_lowering=False, debug=False,
                   num_devices=NCORES)

    xT = nc.dram_tensor("xT", [D, T], BF16, kind="ExternalInput").ap()
    wqT = nc.dram_tensor("wqT", [D, QC], BF16, kind="ExternalInput").ap()
    wkvT = nc.dram_tensor("wkvT", [D, 128], BF16, kind="ExternalInput").ap()
    woT = nc.dram_tensor("woT", [QC, D], BF16, kind="ExternalInput").ap()
    cosT = nc.dram_tensor("cosT", [128, T], F32, kind="ExternalInput").ap()
    sinT = nc.dram_tensor("sinT", [128, T], F32, kind="ExternalInput").ap()
    out = nc.dram_tensor("out", [T, D], BF16, kind="ExternalOutput").ap()
    dbg = {}
    if debug:
        for nm, shp in [("d_qT0", [128, T]), ("d_qT1", [128, T]),
                        ("d_kT", [128, T]), ("d_ytps", [128, S]),
                        ("d_bc", [128, S])]:
            dbg[nm] = nc.dram_tensor(nm, shp, F32, kind="ExternalOutput").ap()

    with tile.TileContext(nc) as tc:
        with (
            tc.tile_pool(name="consts", bufs=1) as consts,
            tc.tile_pool(name="persist", bufs=1) as persist,
            tc.tile_pool(name="xa", bufs=48) as xap,
            tc.tile_pool(name="rtmp", bufs=6) as rtmp,
            tc.tile_pool(name="swp", bufs=4) as swp,
            tc.tile_pool(name="vtmp", bufs=2) as vtmp,
            tc.tile_pool(name="expp", bufs=3) as expp,
            tc.tile_pool(name="ytn", bufs=8) as ytnp,
            tc.tile_pool(name="outst", bufs=4) as outst,
            tc.tile_pool(name="dn", bufs=4) as dnp,
            tc.tile_pool(name="mm", bufs=2, space="PSUM") as mmp,
            tc.tile_pool(name="ytps", bufs=2, space="PSUM") as ytps,
            tc.tile_pool(name="pairp", bufs=2, space="PSUM") as pairp,
        ):
            # identity + warmup junk come first so their gpsimd ops are not
            # queued behind DMA descriptor generation
            ident = consts.tile([128, 128], F32)
            make_identity(nc, ident)
            identb = consts.tile([128, 128], BF16)
            nc.vector.tensor_copy(identb, ident)
            junk = consts.tile([128, 512], F32)
            nc.vector.memset(junk, 1.0)

            # ---- constants; DMA issue order interleaved per k-chunk so the
            # first projection matmul only waits on chunk 0 of wq/x ----
            wq_sb = consts.tile([128, KC, QC], BF16)
            wkv_sb = consts.tile([128, KC, 128], BF16)
            xa_strips = {}

            def load_xa(strip):
                t0 = strip * S
                xa = []
                for kc in range(KC):
                    xt = xap.tile([128, S], BF16, tag="xa", name=f"xa{strip}_{kc}")
                    nc.sync.dma_start(
                        out=xt, in_=xT[kc * 128:(kc + 1) * 128, t0:t0 + S])
                    xa.append(xt)
                xa_strips[strip] = xa

            # startup input DMAs are ISSUE-bound (~600ns per descriptor), so
            # spread them across three otherwise-idle queues
            xa0 = []
            for kc in range(KC):
                nc.scalar.dma_start(
                    out=wq_sb[:, kc, :], in_=wqT[kc * 128:(kc + 1) * 128, :])
                nc.gpsimd.dma_start(
                    out=wkv_sb[:, kc, :], in_=wkvT[kc * 128:(kc + 1) * 128, :])
                xt = xap.tile([128, S], BF16, tag="xa", name=f"xa0_{kc}")
                nc.sync.dma_start(out=xt, in_=xT[kc * 128:(kc + 1) * 128, 0:S])
                xa0.append(xt)
            xa_strips[0] = xa0
            cs_c = consts.tile([128, T], F32)
            cs_s = consts.tile([128, T], F32)
            for n in range(NSTRIP):
                nc.scalar.dma_start(
                    out=cs_c[:, n * S:(n + 1) * S],
                    in_=cosT[:, n * S:(n + 1) * S])
                nc.scalar.dma_start(
                    out=cs_s[:, n * S:(n + 1) * S],
                    in_=sinT[:, n * S:(n + 1) * S])
            wo_sb = consts.tile([128, 2, D], BF16)
            nc.scalar.dma_start(
                out=wo_sb, in_=woT.rearrange("(c p) n -> p c n", p=128))
            # second strip of x prefetched right behind the first; issue
            # slots split so neither queue serializes the whole strip
            xa1 = []
            for kc in range(KC):
                xt = xap.tile([128, S], BF16, tag="xa", name=f"xa1_{kc}")
                eng = nc.sync if kc % 2 == 0 else nc.scalar
                eng.dma_start(out=xt, in_=xT[kc * 128:(kc + 1) * 128, S:2 * S])
                xa1.append(xt)
            xa_strips[1] = xa1
            # PE warmup: sustained array activity lifts the HAM 1.2GHz cold
            # throttle while the first input DMAs land (fp32 on purpose:
            # 4 cyc/row keeps the array busy longer per instruction)
            warm_ps = pairp.tile([128, 1024], F32, tag="pair", name="warm")
            for w in range(6):
                nc.tensor.matmul(
                    warm_ps[:, 0:512], ident, junk,
                    start=True, stop=True, skip_group_check=True)

            # persistent activations
            qT = [persist.tile([128, T], BF16, tag=f"qT{i}", name=f"qT{i}")
                  for i in range(2)]
            # k duplicated on both partition halves so each q head can use
            # a stationary slice whose base partition matches its rhs base
            kT = persist.tile([128, T], BF16)
            vaug = persist.tile([128, 4 * NSTRIP, 65], BF16)
            ones_col = consts.tile([128, 4 * NSTRIP, 1], F32)
            nc.vector.memset(ones_col, 1.0)
            nc.vector.tensor_copy(vaug[:, :, 64:65], ones_col)

            def rope_q(strip, hp, pq):
                t0 = strip * S
                tsl = slice(t0, t0 + S)
                qc = rtmp.tile([128, S], BF16, tag="rtmp", name=f"qc{strip}{hp}")
                qs = rtmp.tile([128, S], BF16, tag="rtmp", name=f"qs{strip}{hp}")
                nc.vector.tensor_mul(qc, pq, cs_c[:, tsl])
                nc.vector.tensor_mul(qs, pq, cs_s[:, tsl])
                sw = swp.tile([128, S], BF16, tag="swp", name=f"sw{strip}{hp}")
                for b in range(2):
                    nc.gpsimd.dma_start(
                        out=sw[b * 64:b * 64 + 32, :],
                        in_=qs[b * 64 + 32:b * 64 + 64, :])
                    nc.gpsimd.dma_start(
                        out=sw[b * 64 + 32:b * 64 + 64, :],
                        in_=qs[b * 64:b * 64 + 32, :])
                nc.vector.tensor_add(qT[hp][:, tsl], qc, sw)

            def rope_kv(strip, pkv):
                t0 = strip * S
                tsl = slice(t0, t0 + S)
                kc_t = rtmp.tile([128, S], BF16, tag="rtmp", name=f"kc{strip}")
                ks_t = rtmp.tile([128, S], BF16, tag="rtmp", name=f"ks{strip}")
                nc.vector.tensor_mul(
                    kc_t[0:64, :], pkv[0:64, :], cs_c[0:64, tsl])
                nc.vector.tensor_mul(
                    ks_t[0:64, :], pkv[0:64, :], cs_s[0:64, tsl])
                swk = swp.tile([128, S], BF16, tag="swp", name=f"swk{strip}")
                nc.gpsimd.dma_start(out=swk[0:32, :], in_=ks_t[32:64, :])
                nc.gpsimd.dma_start(out=swk[32:64, :], in_=ks_t[0:32, :])
                nc.vector.tensor_add(
                    kT[0:64, tsl], kc_t[0:64, :], swk[0:64, :])
                nc.gpsimd.dma_start(out=kT[64:128, tsl], in_=kT[0:64, tsl])
                vt_s = vtmp.tile([128, S], BF16, tag="vtmp", name=f"vt{strip}")
                nc.vector.tensor_copy(vt_s[64:128, :], pkv[64:128, :])
                return vt_s

            def v_transpose(strip, vt_s, n):
                pt = mmp.tile([128, 64], BF16, tag="mmp", bufs=1,
                              name=f"pt{strip}{n}")
                nc.tensor.transpose(
                    pt, vt_s[64:128, n * 128:(n + 1) * 128],
                    identb[64:128, 64:128])
                nc.vector.tensor_copy(vaug[:, strip * 4 + n, 0:64], pt)

            def proj_strip0():
                """Strip 0 runs dense and DMA-paced, so all three projection
                groups interleave per x-chunk (q head-pairs in the idle pair
                banks) and finish right after the last chunk lands."""
                xa = xa_strips[0]
                pq0 = mmp.tile([128, S], F32, tag="mmp", bufs=1, name="s0pq0")
                pq1 = pairp.tile([128, 1024], F32, tag="pair", name="s0pq1")
                pkv = pairp.tile([128, 1024], F32, tag="pair", name="s0pkv")
                for kc in range(KC):
                    nc.tensor.matmul(
                        pq0, wq_sb[:, kc, 0:128], xa[kc],
                        start=(kc == 0), stop=(kc == KC - 1),
                        skip_group_check=True)
                    nc.tensor.matmul(
                        pq1[:, 0:512], wq_sb[:, kc, 128:256], xa[kc],
                        start=(kc == 0), stop=(kc == KC - 1),
                        skip_group_check=True)
                    nc.tensor.matmul(
                        pkv[:, 0:512], wkv_sb[:, kc, :], xa[kc],
                        start=(kc == 0), stop=(kc == KC - 1),
                        skip_group_check=True)
                rope_q(0, 0, pq0)
                rope_q(0, 1, pq1[:, 0:512])
                vt_s = rope_kv(0, pkv[:, 0:512])
                for n in range(4):
                    v_transpose(0, vt_s, n)

            def proj_filler(strip):
                """Yield closures, each emitting one PE op of this strip's
                q/kv projection; rope/evict DVE work rides along after the
                last matmul of each accumulation group."""
                xa = xa_strips[strip]

                for hp in range(2):
                    pq = mmp.tile([128, S], F32, tag="mmp", bufs=1,
                                   name=f"pq{strip}_{hp}")
                    for kc in range(KC):
                        def mk(hp=hp, pq=pq, kc=kc):
                            nc.tensor.matmul(
                                pq, wq_sb[:, kc, hp * 128:(hp + 1) * 128],
                                xa[kc], start=(kc == 0), stop=(kc == KC - 1))
                            if kc == KC - 1:
                                rope_q(strip, hp, pq)
                        yield mk

                pkv = mmp.tile([128, S], F32, tag="mmp", bufs=1,
                                name=f"pkv{strip}")
                state = {}
                for kc in range(KC):
                    def mk(kc=kc):
                        nc.tensor.matmul(
                            pkv, wkv_sb[:, kc, :], xa_strips[strip][kc],
                            start=(kc == 0), stop=(kc == KC - 1))
                        if kc == KC - 1:
                            state["vt_s"] = rope_kv(strip, pkv)
                    yield mk
                for n in range(4):
                    def mk(n=n):
                        v_transpose(strip, state["vt_s"], n)
                    yield mk

            def oproj_filler(strip, ytn):
                """Yield closures, each emitting one o_proj matmul; the
                eviction + store ride along after each group's stop."""
                t0 = strip * S
                for tsub in range(4):
                    trow = t0 + tsub * 128
                    for n in range(4):
                        po = mmp.tile([128, S], F32, tag="mmo", bufs=1,
                                      name=f"po{strip}{tsub}{n}")
                        for c in range(2):
                            def mk(po=po, c=c, tsub=tsub, n=n, trow=trow):
                                nc.tensor.matmul(
                                    po,
                                    ytn[c][:, tsub * 128:(tsub + 1) * 128],
                                    wo_sb[:, c, n * S:(n + 1) * S],
                                    start=(c == 0), stop=(c == 1),
                                    skip_group_check=True)
                                if c == 1:
                                    ot = outst.tile(
                                        [128, S], BF16, tag="out",
                                        name=f"ot{strip}{tsub}{n}")
                                    nc.vector.tensor_copy(ot, po)
                                    nc.sync.dma_start(
                                        out=out[trow:trow + 128,
                                                n * S:(n + 1) * S],
                                        in_=ot)
                            yield mk

            # strip 0 projection runs dense (nothing to overlap with)
            proj_strip0()

            import itertools
            ytn_strips = {}

            # filler queues drained strictly FIFO (they share single-bank
            # PSUM rings, so two open queues must never interleave), paced
            # EDF-style: enough ops per call site that every queue finishes
            # by the end of its deadline strip. Attention is slightly
            # ACT-bound per pair, so rationing PE filler across strips keeps
            # every strip PE-bound and the HAM clock-gate warm.
            # Queue: [iter, remaining, deadline_strip, min_call_index]
            queues = []
            CALLS_PER_STRIP = [HPC * (2 * (s + 1) + 2) for s in range(NSTRIP)]

            def make_queues(strip):
                if strip == 0:
                    queues.append([iter(proj_filler(1)), N_PROJ_OPS, 0, 0])
                    queues.append([iter(proj_filler(2)), N_PROJ_OPS, 1, 8])
                elif strip == 1:
                    queues.append([iter(proj_filler(3)), N_PROJ_OPS, 2, 0])
                    queues.append([iter(oproj_filler(0, ytn_strips[0])),
                                   N_OPROJ_OPS, 2, 6])
                elif strip == 2:
                    queues.append([iter(oproj_filler(1, ytn_strips[1])),
                                   N_OPROJ_OPS, 3, 6])
                elif strip == 3:
                    queues.append([iter(oproj_filler(2, ytn_strips[2])),
                                   N_OPROJ_OPS, 3, 6])

            for strip in range(NSTRIP):
                t0 = strip * S
                n_sc = (strip + 1) * 4
                n_pair = n_sc // 2
                ytn = [ytnp.tile([128, S], BF16, tag="ytn",
                                 name=f"ytn{strip}{i}") for i in range(2)]
                ytn_strips[strip] = ytn

                # x two strips ahead so projection fillers never block the
                # in-order PE on a just-issued DMA
                if strip + 2 < NSTRIP:
                    load_xa(strip + 2)
                make_queues(strip)

                n_calls = CALLS_PER_STRIP[strip]
                ci = {"i": 0}

                def run_fillers(strip=strip, n_calls=n_calls, ci=ci):
                    i = ci["i"]
                    ci["i"] += 1
                    take = 0
                    due = 0
                    for q in sorted(queues, key=lambda q: q[2]):
                        if q[3] > i:
                            continue
                        due += q[1]
                        calls_left = (n_calls - i) + sum(
                            CALLS_PER_STRIP[s] for s in range(strip + 1, q[2] + 1))
                        if calls_left > 0:
                            take = max(take, -(-due // calls_left))
                    while take > 0 and queues:
                        q = queues[0]
                        if q[1] <= 0:
                            queues.pop(0)
                            continue
                        if q[3] > i:
                            break
                        fns = list(itertools.islice(q[0], min(take, q[1])))
                        for fn in fns:
                            fn()
                        q[1] -= len(fns)
                        take -= len(fns)
                        if q[1] <= 0 or not fns:
                            queues.pop(0)

                # even heads (lo=0) first: odd heads need the kT half-dup
                # DMA, which lands a bit after the strip's k-rope
                for h in (0, 2, 1, 3):
                    hp, lo = h // 2, (h % 2) * 64
                    even = (h % 2 == 0)
                    yt_ps = ytps.tile([128, S], F32, tag="yt",
                                      name=f"yt{strip}{h}")

                    def emit_sc(P, h=h, hp=hp, lo=lo):
                        """Scores for chunk pair (2P, 2P+1) into one 2-bank
                        PSUM tile + a single fused exp."""
                        pair = pairp.tile([128, 1024], F32, tag="pair",
                                          name=f"p{strip}{h}{P}")
                        exd = expp.tile([128, 1024], BF16, tag="exp",
                                        name=f"e{strip}{h}{P}")
                        os_ = []
                        for c in range(2):
                            j = P * 2 + c
                            o = max(j * 128 - t0, 0)
                            os_.append(o)
                            nc.tensor.matmul(
                                pair[:, c * 512 + o:(c + 1) * 512],
                                kT[lo:lo + 64, j * 128:(j + 1) * 128],
                                qT[hp][lo:lo + 64, t0 + o:t0 + S],
                                start=True, stop=True)
                        # one exp over the pair; the [512:512+o1) gap holds
                        # stale psum that is exp'd but never read downstream
                        nc.scalar.activation(
                            exd[:, os_[0]:1024], pair[:, os_[0]:1024],
                            mybir.ActivationFunctionType.Exp,
                            scale=1.0 / math.sqrt(HD))
                        for c in range(2):
                            j = P * 2 + c
                            o = os_[c]
                            if j * 128 - t0 >= 0:
                                b = c * 512 + o
                                nc.gpsimd.affine_select(
                                    out=exd[:, b:b + 128],
                                    in_=exd[:, b:b + 128],
                                    pattern=[[1, 128]], base=0,
                                    channel_multiplier=-1,
                                    compare_op=mybir.AluOpType.is_ge, fill=0.0)
                        return exd, os_

                    def emit_av(P, exd, os_, yt_ps=yt_ps, n_pair=n_pair):
                        for c in range(2):
                            j = P * 2 + c
                            o = os_[c]
                            nc.tensor.matmul(
                                yt_ps[0:65, o:S], vaug[:, j, :],
                                exd[:, c * 512 + o:(c + 1) * 512],
                                start=(P == 0 and c == 0),
                                stop=(P == n_pair - 1 and c == 1),
                                skip_group_check=True)

                    prev = None
                    for P in range(n_pair):
                        cur = emit_sc(P)
                        if prev is not None:
                            emit_av(P - 1, *prev)
                            run_fillers()
                        prev = cur
                    emit_av(n_pair - 1, *prev)
                    run_fillers()
                    # normalize: exact 1/denom on the DVE. The denom row is
                    # DMA-spread across 64 lanes first: DVE reciprocal cost
                    # scales with free size (~3.4us on [1,512], ~0.2us on
                    # [64,8]), and ACT would thrash table sets against exp.
                    drow = dnp.tile([128, S], F32, tag="drow", bufs=2,
                                    name=f"drow{strip}{h}")
                    nc.vector.tensor_copy(drow[64:65, :], yt_ps[64:65, :])
                    rs = dnp.tile([128, 8], F32, tag="rs", bufs=2,
                                  name=f"rs{strip}{h}")
                    nc.gpsimd.dma_start(out=rs[0:64, :], in_=drow[64:65, :])
                    rs2 = dnp.tile([128, 8], F32, tag="rs2", bufs=2,
                                   name=f"rs2{strip}{h}")
                    nc.vector.reciprocal(rs2[0:64, :], rs[0:64, :])
                    dn_f = dnp.tile([128, S], F32, tag="dnr", bufs=2,
                                    name=f"dnr{strip}{h}")
                    nc.gpsimd.dma_start(out=dn_f[0:1, :], in_=rs2[0:64, :])
                    nc.gpsimd.dma_start(out=dn_f[32:33, :], in_=rs2[0:64, :])
                    run_fillers()
                    # broadcast p0/p32 across their 32-partition quadrants
                    # with a stream_shuffle (hw DVE op; the gpsimd ucode
                    # partition_broadcast showed a timing-dependent race)
                    bc_t = dnp.tile([128, S], F32, tag="dn", name=f"bc{strip}{h}")
                    nc.vector.stream_shuffle(
                        bc_t[0:64, :], dn_f[0:64, :], mask=[0] * 32)
                    if debug and strip == 0 and h == 0:
                        yd = dnp.tile([128, S], F32, tag="dn", name="yd")
                        nc.vector.tensor_copy(yd, yt_ps)
                        nc.sync.dma_start(out=dbg["d_ytps"], in_=yd)
                        nc.sync.dma_start(out=dbg["d_bc"], in_=bc_t)
                    if even:
                        nc.vector.tensor_mul(
                            ytn[hp][0:64, :], yt_ps[0:64, :], bc_t[0:64, :])
                    else:
                        ntmp = dnp.tile([128, S], BF16, tag="ntmp", bufs=2,
                                        name=f"nt{strip}{h}")
                        nc.vector.tensor_mul(
                            ntmp[0:64, :], yt_ps[0:64, :], bc_t[0:64, :])
                        nc.gpsimd.dma_start(
                            out=ytn[hp][64:128, :], in_=ntmp[0:64, :])

                if debug and strip == 0:
                    nc.sync.dma_start(out=dbg["d_qT0"][:, 0:1024],
                                      in_=qT[0].bitcast(F32)[:, 0:1024])
                    nc.sync.dma_start(out=dbg["d_qT1"][:, 0:1024],
                                      in_=qT[1].bitcast(F32)[:, 0:1024])
                    nc.sync.dma_start(out=dbg["d_kT"][:, 0:1024],
                                      in_=kT.bitcast(F32)[:, 0:1024])

                # drain queues whose deadline is this strip (FIFO order)
                while queues and queues[0][2] <= strip:
                    q = queues.pop(0)
                    for fn in q[0]:
                        fn()

            # last strip's o_proj runs dense at the tail
            for fn in oproj_filler(NSTRIP - 1, ytn_strips[NSTRIP - 1]):
                fn()

    nc.compile()
    return nc


_NC_CACHE = None


def _get_nc():
    global _NC_CACHE
    if _NC_CACHE is None:
        _NC_CACHE = _build_kernel()
    return _NC_CACHE


def _prep_inputs(x, wq, wk, wv, wo):
    """Host-side shard + layout prep. Returns per-core input maps."""
    import ml_dtypes
    BF = ml_dtypes.bfloat16
    x = np.asarray(x, dtype=np.float32).reshape(T, D)
    wq = np.asarray(wq, dtype=np.float32)
    wk = np.asarray(wk, dtype=np.float32)
    wv = np.asarray(wv, dtype=np.float32)
    wo = np.asarray(wo, dtype=np.float32)

    xT_b = np.ascontiguousarray(x.T).astype(BF)

    # head-dim permutation for rope: [even pair comps | odd pair comps]
    perm = np.concatenate([np.arange(0, HD, 2), np.arange(1, HD, 2)])

    # rope tables in the [d, t] layout
    theta = 1.0 / ROPE_BASE ** (np.arange(0, HD, 2, dtype=np.float64) / HD)
    ang = np.arange(T, dtype=np.float64)[None, :] * theta[:, None]  # [32, T]
    cos_blk = np.cos(ang).astype(np.float32)
    sin_blk = np.sin(ang).astype(np.float32)
    cosT = np.tile(np.concatenate([cos_blk, cos_blk], 0), (2, 1))
    sinT = np.tile(np.concatenate([sin_blk, -sin_blk], 0), (2, 1))
    cosT = np.ascontiguousarray(cosT)
    sinT = np.ascontiguousarray(sinT)

    in_maps = []
    for c in range(NCORES):
        wq_c = wq[c * QC:(c + 1) * QC].reshape(HPC, HD, D)[:, perm, :]
        wq_c = wq_c.reshape(QC, D)
        wk_c = wk[c * HD:(c + 1) * HD][perm, :]
        wv_c = wv[c * HD:(c + 1) * HD]
        wkv_c = np.concatenate([wk_c, wv_c], axis=0)          # [128, D]
        wo_c = wo[:, c * QC:(c + 1) * QC]                      # [D, QC]
        in_maps.append({
            "xT": xT_b,
            "wqT": np.ascontiguousarray(wq_c.T).astype(BF),
            "wkvT": np.ascontiguousarray(wkv_c.T).astype(BF),
            "woT": np.ascontiguousarray(wo_c.T).astype(BF),
            "cosT": cosT,
            "sinT": sinT,
        })
    return in_maps


def kernel(x, wq, wk, wv, wo):
    from concourse.bass_utils import run_bass_kernel_spmd

    nc = _get_nc()
    in_maps = _prep_inputs(x, wq, wk, wv, wo)
    res = run_bass_kernel_spmd(nc, in_maps, core_ids=list(range(NCORES)))
    acc = np.zeros((T, D), dtype=np.float64)
    for c in range(NCORES):
        acc += res.results[c]["out"].astype(np.float64)
    return acc.astype(np.float32).reshape(1, T, D)
